# revision 87
# baseline (speedup 1.0000x reference)
"""AdaptiveGraphConv Trainium2 kernel — 8-core batch-parallel Bass/Tile.

Math (per sample n):
  xm     = mean_t x[n]                                  [C, V]
  theta  = W_theta @ xm + b_theta ; phi similarly       [E, V]
  Cmat   = softmax_w(theta^T @ phi)                     [V, V]
  adap_s = A[s] + B[s] + Cmat                           [V, V]
  out[n] = sum_s W_eff_s @ x[n] @_v adap_s + b_eff      [Co, T, V]
where W_eff_s[co,c] = sum_sg W_big[sg*Co+co, s*C+c], b_eff = sum_sg b_big[sg*Co:+Co]
(both reductions done on device).

Device dataflow (per core, 4 samples). T*V is split into 60 chunks of
(5t, 25v) = 125 elements; a 126th "bias" column per chunk (host-packed,
1.0 at channel 64) carries b_eff through both matmul steps:
  step1: matmul(lhsT = x chunk [c=65, m=126], rhs = Weff_cat [65, 192])
         -> y chunk [(5t,25v)+bias, (s,co)] in PSUM, groups of 5 chunks
         -> y_sb bf16 (plain slice copies on ACT/DVE/Pool)
  mean : accumulating matmul rhs=I64 over all chunks -> x^T sums
  tiny : mean/theta/phi/sim/softmax -> bd3 [126, 3*125] blockdiag bf16
         (bias row 125 = ones in the s=0 block)
  step2: matmul(lhsT = bd3 s-block [126,125], rhs = y chunk [126, 64co])
         accumulated over s, groups of 8 chunks -> [125, 64] -> o_sb bf16
         -> DMA out (host upcasts)
"""

import numpy as np
import ml_dtypes

N, C, T, V, S, E, Co = 32, 64, 300, 25, 3, 64, 64
CP = C + 1                # 65 = x channels + bias channel
NCORES = 8
NL = N // NCORES          # samples per core = 4
CH = 126                  # chunk partitions: (5t, 25v) + bias column
NCH = 60                  # chunks per sample (T/5)
W2 = S * Co               # 192 = y columns per chunk
XW = NCH * CH             # 7560 = x free size per sample
G1 = 4                    # step1 chunks per psum group (15 groups)
G2 = 8                    # step2 chunks per psum group (7 full + 1 of 4)
CF = 1992                 # packed consts free size

_CACHE = {}


def _import_concourse():
    try:
        import concourse  # noqa: F401
    except ImportError:
        import sys

        for p in ("/opt/trn_rl_repo", "/root/.axon_site/_ro/trn_rl_repo"):
            if p not in sys.path:
                sys.path.insert(0, p)


def _build_nc():
    _import_concourse()
    import concourse.bass as bass
    import concourse.bacc as bacc
    import concourse.mybir as mybir
    from concourse import tile

    dt = mybir.dt
    f32, bf16 = dt.float32, dt.bfloat16
    AX = mybir.AxisListType
    ALU = mybir.AluOpType
    ACTF = mybir.ActivationFunctionType

    nc = bacc.Bacc(None, target_bir_lowering=False)

    x_ext = nc.declare_dram_parameter("x", [NL, CP, XW], bf16, isOutput=False)
    c_ext = nc.declare_dram_parameter("consts", [CH, CF], bf16, isOutput=False)
    out_ext = nc.declare_dram_parameter(
        "out", [NL, 125, NCH * Co], bf16, isOutput=True
    )

    with tile.TileContext(nc) as tc:
        with (
            tc.tile_pool(name="const", bufs=1) as cpool,
            tc.tile_pool(name="xin", bufs=2) as xpool,
            tc.tile_pool(name="y", bufs=2) as ypool,
            tc.tile_pool(name="osb", bufs=2) as opool,
            tc.tile_pool(name="small", bufs=2) as spool,
            tc.tile_pool(name="p1", bufs=2, space="PSUM") as pq,
            tc.tile_pool(name="p2", bufs=2, space="PSUM") as po,
            tc.tile_pool(name="pxs", bufs=1, space="PSUM") as pxs,
            tc.tile_pool(name="ptiny", bufs=1, space="PSUM") as pt,
        ):
            # ---------------- PE p-state warmup ----------------
            # ~72 tiny matmuls bridge the DMA/weight-prep startup so the PE
            # hits its 3us continuous-busy ramp before real work arrives.
            wz = cpool.tile([1, Co], bf16)
            nc.gpsimd.memset(wz[:, :], 0.0)
            # tt: single psum bank shared by warmup + all tiny matmul outs
            tt = pt.tile([CH, 301], f32, tag="tt")
            for _ in range(80):
                nc.tensor.matmul(
                    out=tt[0:1, 0:Co], lhsT=wz[:, 0:1], rhs=wz[:, :],
                    start=True, stop=True,
                )

            # ---------------- constants / weight prep ----------------
            ct = cpool.tile([CH, CF], bf16)
            nc.sync.dma_start(out=ct[0:CP, 0:832], in_=c_ext[0:CP, 0:832])
            # part 2 (obd/bg3/A+B, first needed ~14us in) is issued after
            # sample 0's x slices so it doesn't delay them on the DMA rings
            def consts_p2():
                nc.sync.dma_start(out=ct[:, 832:CF], in_=c_ext[:, 832:CF])
                nc.vector.tensor_copy(out=wtp[:, :], in_=wtpf)
                nc.vector.tensor_copy(out=obd[:, :], in_=obdf)
                nc.vector.tensor_copy(out=selc[:, :], in_=self_f)
                nc.vector.tensor_copy(out=id25f[:, :], in_=ct[0:V, 576:601])
                nc.vector.tensor_copy(out=btpf[:, :], in_=ct[0:E, 960:962])
            wbp = ct[0:C, 0:576]
            id64f = ct[0:C, 576:640]
            bbrow = ct[0:1, 640:832]
            wtpf = ct[0:C, 832:960]
            btt = ct[0:E, 960:961]
            btf = ct[0:E, 961:962]
            obdf = ct[0:CH, 962:987]
            self_f = ct[0:V, 987:1617]
            bgab3f = ct[0:CH, 1617:1992]

            wstack = cpool.tile([CP, W2 + C], bf16)
            nc.gpsimd.memset(wstack[C : C + 1, :], 0.0)
            weff_t = cpool.tile([C, W2], f32)
            nc.vector.tensor_tensor(
                out=weff_t[:, :], in0=wbp[:, 0:192], in1=wbp[:, 192:384], op=ALU.add
            )
            # second add writes the bf16 wstack directly (off the critical
            # path copy); id64 cast runs on Pool in parallel
            nc.vector.tensor_tensor(
                out=wstack[0:C, 0:192], in0=weff_t[:, :],
                in1=wbp[:, 384:576], op=ALU.add,
            )
            nc.gpsimd.tensor_copy(out=wstack[0:C, 192:256], in_=id64f)

            befff = cpool.tile([1, Co], f32)
            nc.gpsimd.tensor_tensor(
                out=befff[:, :], in0=bbrow[:, 0:64], in1=bbrow[:, 64:128], op=ALU.add
            )
            # final add writes the bias row in place (partition 64 is
            # 32-aligned, so a partition-shifted engine write is legal)
            nc.gpsimd.tensor_tensor(
                out=wstack[C : C + 1, 0:Co], in0=befff[:, :],
                in1=bbrow[:, 128:192], op=ALU.add,
            )

            wtp = cpool.tile([C, 2 * E], bf16)
            obd = cpool.tile([CH, V], bf16)
            selc = cpool.tile([V, 5 * CH], bf16)
            id25f = cpool.tile([V, V], f32)
            btpf = cpool.tile([E, 2], f32)

            # ---------------- per-sample phases ----------------
            XTLAG = 4  # xtp matmuls trail y matmuls by this many groups

            def phase_a_start(n):
                """Allocate tiles + x DMA for sample n. Sample 0 is on the
                critical path: its later slices issue from the (idle) ACT
                hwdge queue in parallel with SP so the issue rate doesn't
                gate delivery."""
                x_sb = xpool.tile([CP, XW], bf16, tag="x")
                if n == 0:
                    cuts = [0, 6 * CH, 20 * CH, 40 * CH, XW]
                    qs = [nc.sync, nc.sync, nc.scalar, nc.scalar]
                else:
                    cuts = [0, 30 * CH, XW]
                    qs = [nc.sync, nc.sync]
                for q, (lo, hi) in zip(qs, zip(cuts, cuts[1:])):
                    q.dma_start(
                        out=x_sb[:, lo:hi], in_=x_ext[n][:, lo:hi]
                    )
                y_sb = ypool.tile([CH, NCH * W2], bf16, tag="y")
                xtp = pxs.tile([CH, C], f32, tag="xt")
                return {"x": x_sb, "y": y_sb, "xtp": xtp, "n": n}

            def emit_xt(ctx, lo, hi):
                """x^T accumulation matmuls for chunks [lo, hi)."""
                x_sb, xtp = ctx["x"], ctx["xtp"]
                for ch in range(lo, hi):
                    nc.tensor.matmul(
                        out=xtp[:, :],
                        lhsT=x_sb[:, ch * CH : (ch + 1) * CH],
                        rhs=wstack[:, W2 : W2 + C],
                        start=(ch == 0),
                        stop=(ch == NCH - 1),
                    )

            def emit_group(ctx, g, xt="lag", mid=None):
                """step1 psum group g: 4 y-matmuls (+ lagged xt), 1 copy.
                A matmul out may not cross a 512-f32 psum bank boundary, so
                chunk j sits at column (j//2)*512 + (j%2)*192."""
                x_sb, y_sb = ctx["x"], ctx["y"]
                yp = pq.tile([CH, 1024], f32, tag="p1")
                for j in range(G1):
                    ch = G1 * g + j
                    col = (j // 2) * 512 + (j % 2) * W2
                    nc.tensor.matmul(
                        out=yp[:, col : col + W2],
                        lhsT=x_sb[:, ch * CH : (ch + 1) * CH],
                        rhs=wstack[:, 0:W2],
                        start=(j % 2 == 0),
                        stop=(j % 2 == 1),
                    )
                if xt == "lag":
                    if g >= XTLAG:
                        emit_xt(ctx, (g - XTLAG) * G1, (g - XTLAG + 1) * G1)
                    if g == 14:
                        emit_xt(ctx, (15 - XTLAG) * G1, NCH)
                if mid is not None:
                    mid()  # latency-critical ops enqueue ahead of the copy
                dst = y_sb[
                    :, g * G1 * W2 : (g + 1) * G1 * W2
                ].rearrange("p (b w) -> p b w", w=2 * W2)
                src = yp[:, :].rearrange("p (b w) -> p b w", w=512)[
                    :, :, 0 : 2 * W2
                ]
                # GPSIMD cannot access PSUM (walrus birverifier rule), so
                # psum->sbuf copies are split across ACT and DVE only.
                if g % 2 == 0:  # 8 on ACT
                    nc.scalar.copy(out=dst, in_=src)
                else:  # 7 on DVE
                    nc.vector.tensor_copy(out=dst, in_=src)

            def tiny_steps(ctx):
                """Mean/softmax/bd3 chain as 6 steps; weave each between PE
                group emissions so cross-engine hops never stall the PE."""
                n = ctx["n"]

                def s0():  # xta copy (DVE)
                    xta_sb = spool.tile([CH, C], bf16, tag="xta")
                    nc.vector.tensor_copy(out=xta_sb[:, :], in_=ctx["xtp"])
                    ctx["xta"] = xta_sb

                def s1():  # T-sum matmul, then scale by 1/T -> mean
                    xsp = tt[0:V, 0:C]
                    nc.tensor.matmul(
                        out=xsp, lhsT=obd[:, :], rhs=ctx["xta"][:, :],
                        start=True, stop=True,
                    )
                    xs_sb = spool.tile([V, C], f32, tag="xs_sb")
                    nc.scalar.activation(
                        out=xs_sb[:, :], in_=xsp, func=ACTF.Copy,
                        scale=1.0 / T,
                    )
                    ctx["xs"] = xs_sb

                def s2():  # transpose to [c, v]
                    xmt = tt[0:C, 64:89]
                    nc.tensor.transpose(
                        out=xmt, in_=ctx["xs"], identity=id25f[:, :]
                    )
                    xm_sb = spool.tile([C, V], bf16, tag="xm_sb")
                    nc.vector.tensor_copy(out=xm_sb[:, :], in_=xmt)
                    ctx["xm"] = xm_sb

                def s3():  # theta / phi
                    thp = tt[0:E, 89:114]
                    nc.tensor.matmul(
                        out=thp, lhsT=wtp[:, 0:E], rhs=ctx["xm"][:, :],
                        start=True, stop=True,
                    )
                    php = tt[0:E, 114:139]
                    nc.tensor.matmul(
                        out=php, lhsT=wtp[:, E : 2 * E],
                        rhs=ctx["xm"][:, :], start=True, stop=True,
                    )
                    th_sb = spool.tile([E, V], bf16, tag="th_sb")
                    nc.scalar.activation(
                        out=th_sb[:, :], in_=thp, func=ACTF.Identity,
                        bias=btpf[:, 0:1],
                    )
                    ph_sb = spool.tile([E, V], bf16, tag="ph_sb")
                    nc.vector.tensor_scalar(
                        out=ph_sb[:, :], in0=php,
                        scalar1=btpf[:, 1:2], scalar2=None, op0=ALU.add,
                    )
                    ctx["th"], ctx["ph"] = th_sb, ph_sb

                def s4():  # sim = theta^T @ phi
                    simp = tt[0:V, 139:164]
                    nc.tensor.matmul(
                        out=simp, lhsT=ctx["th"][:, :],
                        rhs=ctx["ph"][:, :], start=True, stop=True,
                    )
                    ctx["simp"] = simp

                def s5():  # softmax -> cmb (bf16); row sums fused into Exp
                    ex = spool.tile([V, V], f32, tag="ex")
                    rs = spool.tile([V, 1], f32, tag="rs")
                    nc.scalar.activation(
                        out=ex[:, :], in_=ctx["simp"], func=ACTF.Exp,
                        accum_out=rs[:, :],
                    )
                    rr = spool.tile([V, 1], f32, tag="rr")
                    nc.vector.reciprocal(out=rr[:, :], in_=rs[:, :])
                    cmb = spool.tile([V, V], bf16, tag="cm")
                    nc.gpsimd.tensor_scalar(
                        out=cmb[:, :], in0=ex[:, :],
                        scalar1=rr[:, 0:1], scalar2=None, op0=ALU.mult,
                    )
                    ctx["cmb"] = cmb

                def s6():  # blockdiag(cm) via 5 selector matmuls, then
                    # bd3 = bgab3 (static A+B blockdiags + bias row) + tiled
                    # blockdiag(cm) in one wide vector op.
                    bdcmp = tt[0:CH, 176:301]
                    for tau in range(5):
                        nc.tensor.matmul(
                            out=bdcmp[:, 25 * tau : 25 * tau + 25],
                            lhsT=selc[:, tau * CH : (tau + 1) * CH],
                            rhs=ctx["cmb"][:, :],
                            start=True, stop=True,
                        )
                    bd3 = spool.tile([CH, S * 125], bf16, tag="bd3")
                    nc.vector.tensor_tensor(
                        out=bd3[:, :].rearrange("p (s w) -> p s w", w=125),
                        in0=bgab3f.rearrange("p (s w) -> p s w", w=125),
                        in1=bdcmp[:, None, :].broadcast_to([CH, S, 125]),
                        op=ALU.add,
                    )
                    ctx["bd3"] = bd3

                return [s0, s1, s2, s3, s4, s5, s6]

            def phase_a_groups(ctx, weave=None):
                """Emit all 15 step1 groups, weaving tiny steps of the
                previous sample between the early groups."""
                weave = dict(weave or {})
                for g in range(15):
                    emit_group(ctx, g)
                    if g in weave:
                        weave[g]()

            def phase_b_state(n, fine_tail=False, flip=False):
                o_sb = opool.tile([125, NCH * Co], bf16, tag="o")
                # (group_size, copy_engine); engines: a=ACT, v=DVE, p=Pool
                if flip:
                    plan = [(8, "v"), (8, "a"), (8, "v"), (8, "a"), (8, "v"),
                            (8, "a"), (8, "v")]
                else:
                    plan = [(8, "a"), (8, "v"), (8, "a"), (8, "v"), (8, "a"),
                            (8, "v"), (8, "a")]
                plan += ([(2, "v"), (1, "a"), (1, "v")] if fine_tail
                         else [(4, "v")])
                dmas = {2: (0, 24), 5: (24, 48)}
                if fine_tail:
                    dmas.update({6: (48, 56), 9: (56, 60)})
                else:
                    dmas.update({7: (48, 60)})
                ch0s, c = [], 0
                for nch, _ in plan:
                    ch0s.append(c)
                    c += nch
                return {"n": n, "o": o_sb, "plan": plan, "dmas": dmas,
                        "ch0s": ch0s}

            def phase_b_group(bs, ctx, g):
                """step2 psum group g: s-accumulated matmuls + copy + DMA."""
                n, o_sb = bs["n"], bs["o"]
                y_sb, bd3 = ctx["y"], ctx["bd3"]
                nch, eng = bs["plan"][g]
                ch0 = bs["ch0s"][g]
                op = po.tile([CH, G2 * Co], f32, tag="p2")
                for s in range(S):
                    for j in range(nch):
                        ch = ch0 + j
                        nc.tensor.matmul(
                            out=op[0:125, j * Co : (j + 1) * Co],
                            lhsT=bd3[:, s * 125 : (s + 1) * 125],
                            rhs=y_sb[:, ch * W2 + s * Co : ch * W2 + (s + 1) * Co],
                            start=(s == 0 and j == 0),
                            stop=(s == S - 1 and j == nch - 1),
                        )
                dst = o_sb[:, ch0 * Co : (ch0 + nch) * Co]
                src = op[0:125, 0 : nch * Co]
                if eng == "a":
                    nc.scalar.copy(out=dst, in_=src)
                else:
                    nc.vector.tensor_copy(out=dst, in_=src)
                if g in bs["dmas"]:
                    lo, hi = bs["dmas"][g]
                    # the very last piece issues from the (idle) ACT queue so
                    # it doesn't serialize behind the previous SP issue
                    q = nc.scalar if g == 9 else nc.sync
                    q.dma_start(
                        out=out_ext[n][:, lo * Co : hi * Co],
                        in_=o_sb[:, lo * Co : hi * Co],
                    )

            # pipeline. Each round n: step1 groups of sample n, with
            # sample n's x^T matmuls early (g1-g6, x was prefetched last
            # round), its tiny chain at g7-g13, and the PREVIOUS sample's
            # step2 groups at every other position. This keeps the PE fed
            # while every psum->sbuf copy gets ~3 group-times of runway.
            ctxs = [phase_a_start(0)]
            consts_p2()
            st = None

            # round 0: xt woven 6 chunks/group for uniform PE pacing
            for g in range(15):
                if g >= 11 and st is None:
                    st = tiny_steps(ctxs[0])
                emit_group(ctxs[0], g, xt="none",
                           mid=st[g - 11] if g >= 11 else None)
                if 1 <= g <= 10:
                    emit_xt(ctxs[0], (g - 1) * 6, g * 6)
                if g == 8:
                    ctxs.append(phase_a_start(1))
            for k in (4, 5, 6):
                st[k]()
            for n in (1, 2):
                bs = phase_b_state(n - 1)
                st = None
                for g in range(15):
                    if g >= 7 and st is None:
                        st = tiny_steps(ctxs[n])
                    midf = None
                    if 7 <= g <= 13:
                        midf = st[g - 7]
                    emit_group(ctxs[n], g, xt="none", mid=midf)
                    if 1 <= g <= 6:
                        emit_xt(ctxs[n], (g - 1) * 10, g * 10)
                    if g == 8 and n == 1:
                        ctxs.append(phase_a_start(2))
                    if g == 0 and n == 2:
                        ctxs.append(phase_a_start(3))
                    if n == 2 and g >= 8:
                        # sample 3's x^T matmuls run here so its tiny chain
                        # can start at round 3 g0 and B3 can interleave
                        lo = (g - 8) * 9
                        emit_xt(ctxs[3], lo, min(lo + 9, NCH))
                    if g in (2, 4, 6, 8, 10, 12, 14):
                        phase_b_group(bs, ctxs[n - 1], g // 2 - 1)
                phase_b_group(bs, ctxs[n - 1], 7)
            # round 3: T3 at g0-6; B2 and B3's first groups share positions
            bs2 = phase_b_state(2)
            bs3 = phase_b_state(3, fine_tail=True)
            st3 = tiny_steps(ctxs[3])
            sched = {2: [(bs2, 2, 0)], 4: [(bs2, 2, 1)], 6: [(bs2, 2, 2)],
                     7: [(bs2, 2, 3)], 8: [(bs3, 3, 0)], 9: [(bs2, 2, 4)],
                     10: [(bs3, 3, 1)], 11: [(bs2, 2, 5)],
                     12: [(bs3, 3, 2), (bs2, 2, 6)],
                     13: [(bs3, 3, 3), (bs2, 2, 7)],
                     14: [(bs3, 3, 4), (bs3, 3, 5), (bs3, 3, 6)]}
            for g in range(15):
                midf = st3[g] if g <= 6 else None
                emit_group(ctxs[3], g, xt="none", mid=midf)
                for bsx, cn, j in sched.get(g, []):
                    phase_b_group(bsx, ctxs[cn], j)
            for j in range(7, len(bs3["plan"])):
                phase_b_group(bs3, ctxs[3], j)

    nc.finalize()
    return nc


def _prep_consts(A, B, W_theta, b_theta, W_phi, b_phi, W_big, b_big):
    f = np.float32
    ct = np.zeros((CH, CF), dtype=f)  # filled in f32, cast to bf16 at return
    ct[0:C, 0:576] = (
        W_big.reshape(S, Co, S, C).transpose(3, 0, 2, 1).reshape(C, 3 * S * Co)
    )
    ct[0:C, 576:640] = np.eye(C, dtype=f)
    ct[0:1, 640:832] = b_big.reshape(1, S * Co)
    ct[0:C, 832:960] = np.concatenate([W_theta.T, W_phi.T], axis=1)
    ct[0:E, 960] = b_theta
    ct[0:E, 961] = b_phi
    ct[0:125, 962:987] = np.tile(np.eye(V, dtype=f), (5, 1))
    # selector lhsTs: sel_tau[v, p] = 1 iff p == 25*tau + v
    for tau in range(5):
        for v in range(V):
            ct[v, 987 + tau * CH + 25 * tau + v] = 1.0
    # bd3 background: blockdiag(A_s+B_s) per s-block + bias row in s=0
    AB = (A + B).astype(f)
    for sb in range(S):
        for tau in range(5):
            r0, c0 = 25 * tau, 1617 + 125 * sb + 25 * tau
            ct[r0 : r0 + 25, c0 : c0 + 25] = AB[sb]
    ct[125, 1617:1742] = 1.0
    return {"consts": ct.astype(ml_dtypes.bfloat16)}


def _prep_x(x):
    bf = ml_dtypes.bfloat16
    xp = np.zeros((N, CP, NCH, CH), dtype=bf)
    xp[:, :C, :, :125] = x.reshape(N, C, NCH, 125).astype(bf)
    xp[:, C, :, 125] = 1.0  # bias column per chunk
    return xp.reshape(N, CP, XW)


def kernel(x, A, B, W_theta, b_theta, W_phi, b_phi, W_big, b_big, _profile=None):
    _import_concourse()
    from concourse.bass_utils import run_bass_kernel_spmd

    x = np.asarray(x, dtype=np.float32)
    xp = _prep_x(x)

    consts = _prep_consts(
        np.asarray(A, np.float32), np.asarray(B, np.float32),
        np.asarray(W_theta, np.float32), np.asarray(b_theta, np.float32),
        np.asarray(W_phi, np.float32), np.asarray(b_phi, np.float32),
        np.asarray(W_big, np.float32), np.asarray(b_big, np.float32),
    )

    if "nc" not in _CACHE:
        _CACHE["nc"] = _build_nc()
    nc = _CACHE["nc"]

    in_maps = []
    for i in range(NCORES):
        m = {"x": np.ascontiguousarray(xp[i * NL : (i + 1) * NL])}
        m.update(consts)
        in_maps.append(m)

    kw = {}
    if _profile:
        kw = dict(trace=True, tmpdir=_profile)
    res = run_bass_kernel_spmd(nc, in_maps, list(range(NCORES)), **kw)

    out = np.empty((N, Co, T, V), dtype=np.float32)
    for i in range(NCORES):
        buf = np.asarray(res.results[i]["out"], dtype=np.float32).reshape(
            NL, 5, V, NCH, Co
        )
        # [n, tau, w, ch, co] -> [n, co, ch, tau, w]
        out[i * NL : (i + 1) * NL] = (
            buf.transpose(0, 4, 3, 1, 2).reshape(NL, Co, T, V)
        )
    if _profile:
        _CACHE["exec_time_ns"] = res.exec_time_ns
    return out
